# revision 1
# baseline (speedup 1.0000x reference)
"""Trainium2 Bass kernel for nn_LogicLayer (soft logic-gate layer).

Math:
  pA = softmax(Wa, axis=1); pB = softmax(Wb, axis=1); pT = softmax(tw, axis=0)
  a = pA @ X ; b = pB @ X ; out = sum_g pT[g] * gate_g(a, b)

Each of the 16 soft gates is affine in {1, A, B, A*B}, so with C[g, :] =
(c1, cA, cB, cAB) per gate:
  out = w1 + wA*a + wB*b + wAB*(a*b),   w_j[m] = sum_g pT[g, m] * C[g, j]
All softmax normalizers fold into the coefficients (computed on device
from unnormalized exp sums), so the main loop is two fp8 DoubleRow
matmul groups plus a 4-op elementwise epilogue per tile. DoubleRow
contracts two 128-row k-blocks per instruction at the fp8 rate, halving
PE time vs bf16 (874us -> ~440us for the full batch on one core). The
row sums are computed from the SAME fp8-quantized exp values, so the
softmax renormalization cancels most of the quantization error.

Distribution / I/O design. Measurements in this environment show the
per-execution wall time through the axon PJRT path is dominated by
per-execute overhead, not device time: a fixed ~3ms dispatch floor per
NeuronCore-group call, plus input buffer bytes re-staged (h2d) on every
execution, with on-device time adding roughly 1:1 on top. Sharding 8
ways multiplies the dispatch overhead (~+0.5-0.7ms/core) and re-ships
the replicated weights per core, so the fastest configuration is a
SINGLE core with minimal bytes:

  - xin fp8 [1024, NB]: x quantized to fp8e4 on host (16MB) — exactly
    the quantization the DoubleRow matmuls need, so no on-device
    conversion pass;
  - wtw bf16 [1024, 2*1024+16] = [ Wa^T | Wb^T | tw^T ] (4MB);
  - out bf16 [1024, NB] carries NO initializing operand: the kernel
    writes every element, so the result buffer needs no zero-fill and
    no output bytes are uploaded per execution.
  - host converts the returned bf16 result back to f32.

Accuracy: fp8 matmul operands + bf16 weights/output give max rel err
~4.9e-3 (gate 2e-2; CPU emulation, CoreSim, and HW all agree). The tw^T
columns are un-transposed on device via PE identity-matmul transposes;
the coefficient math stays f32.
"""

import sys

if "/opt/trn_rl_repo" not in sys.path:
    sys.path.insert(0, "/opt/trn_rl_repo")

import numpy as np
import ml_dtypes

import concourse.bass as bass
import concourse.mybir as mybir
import concourse.tile as tile

N_CORES = 1
SIZE = 1024
PREV = 1024
BATCH = 16384
NT = 512               # n-tile (one PSUM bank of f32)
KB = PREV // 128       # 8 k-blocks
MB = SIZE // 128       # 8 m-blocks

F32 = mybir.dt.float32
BF16 = mybir.dt.bfloat16
FP8 = mybir.dt.float8e4
NP_BF16 = ml_dtypes.bfloat16

# Gate coefficient matrix: columns = [const, A, B, AB, ones]; rows = gate id.
_C16 = np.array(
    [
        # 1   A   B  AB  ones
        [0,  0,  0,  0, 1],  # 0  FALSE
        [0,  0,  0,  1, 1],  # 1  A AND B
        [0,  1,  0, -1, 1],  # 2  A AND NOT B
        [0,  1,  0,  0, 1],  # 3  A
        [0,  0,  1, -1, 1],  # 4  NOT A AND B
        [0,  0,  1,  0, 1],  # 5  B
        [0,  1,  1, -2, 1],  # 6  XOR
        [0,  1,  1, -1, 1],  # 7  OR
        [1, -1, -1,  1, 1],  # 8  NOR
        [1, -1, -1,  2, 1],  # 9  XNOR
        [1,  0, -1,  0, 1],  # 10 NOT B
        [1,  0, -1,  1, 1],  # 11 B -> A
        [1, -1,  0,  0, 1],  # 12 NOT A
        [1, -1,  0,  1, 1],  # 13 A -> B
        [1,  0,  0, -1, 1],  # 14 NAND
        [1,  0,  0,  0, 1],  # 15 TRUE
    ],
    dtype=np.float32,
)


def _split_waits(nc, maxw=1):
    """Walrus in this container encodes at most one sync-wait per
    instruction; hoist excess waits into preceding NoOps on the same
    engine (semantically an AND of waits, executed in sequence)."""
    for f in nc.m.functions:
        for blk in f.blocks:
            new_list = []
            changed = False
            for inst in blk.instructions:
                si = inst.sync_info
                if si is not None and len(si.on_wait) > maxw:
                    waits = list(si.on_wait)
                    chunks = [waits[i : i + maxw] for i in range(0, len(waits), maxw)]
                    for ci, ch in enumerate(chunks[:-1]):
                        nop = mybir.InstNoOp(
                            name=f"{inst.name}-wsplit{ci}", ins=[], outs=[]
                        )
                        nop.engine = inst.engine
                        nop.sync_info = mybir.SyncInfo(on_wait=ch, on_update=[])
                        new_list.append(nop)
                    inst.sync_info = mybir.SyncInfo(
                        on_wait=chunks[-1], on_update=list(si.on_update)
                    )
                    changed = True
                new_list.append(inst)
            if changed:
                blk.instructions = new_list


def build_nc(n_cores=N_CORES, reps=1):
    # reps>1 repeats the main loop inside the NEFF (timing only: slope
    # between reps values isolates in-NEFF kernel time from the axon
    # dispatch floor).
    nb = BATCH // n_cores
    n_nt = nb // NT

    nc = bass.Bass()
    xin_d = nc.dram_tensor("xin", [PREV, nb], FP8, kind="ExternalInput")
    wtw_d = nc.dram_tensor("wtw", [PREV, 2 * SIZE + 16], BF16, kind="ExternalInput")
    out_d = nc.dram_tensor("out", [SIZE, nb], BF16, kind="ExternalOutput")
    c16_d = nc.inline_tensor(_C16, "c16")
    ident_d = nc.inline_tensor(np.eye(128, dtype=np.float32), "ident")

    AF = mybir.ActivationFunctionType
    OP = mybir.AluOpType

    with tile.TileContext(nc) as tc:
        with (
            tc.tile_pool(name="persist", bufs=1) as pp,
            tc.tile_pool(name="wstage", bufs=3) as wstage,
            tc.tile_pool(name="xbuf", bufs=2) as xbuf,
            tc.tile_pool(name="epi", bufs=3) as epi,
            tc.tile_pool(name="outp", bufs=4) as outp,
            tc.tile_pool(name="psum", bufs=2, space="PSUM") as psp,
            tc.tile_pool(name="psum1", bufs=1, space="PSUM") as psp1,
        ):
            # --- constants ---
            c16s = pp.tile([16, 5], F32, tag="c16s", name="c16s")
            nc.sync.dma_start(out=c16s, in_=c16_d[:, :])
            ones = pp.tile([128, 1], FP8, tag="ones", name="ones")
            nc.vector.memset(ones, 1.0)
            identf = pp.tile([128, 128], F32, tag="identf", name="identf")
            nc.sync.dma_start(out=identf, in_=ident_d[:, :])
            ident = pp.tile([128, 128], BF16, tag="ident", name="ident")
            nc.vector.tensor_copy(ident, identf)

            # --- table coefficients: un-transpose tw^T from wtw, then exp ---
            et = pp.tile([16, SIZE], F32, tag="et", name="et")
            twc = 2 * SIZE
            for mb in range(MB):
                ms = slice(mb * 128, (mb + 1) * 128)
                twt = wstage.tile([128, 16], BF16, tag="twt", name="twt")
                nc.sync.dma_start(out=twt, in_=wtw_d[ms, twc : twc + 16])
                ptw = psp1.tile([16, 128], BF16, tag="ptw", name="ptw")
                nc.tensor.transpose(ptw, twt, ident)
                nc.scalar.activation(et[:, ms], ptw, AF.Exp)
            # fp32 PE matmuls only carry ~bf16 precision, so split et into
            # bf16 hi+lo and accumulate two exact bf16 matmuls.
            c16b = pp.tile([16, 5], BF16, tag="c16b", name="c16b")
            nc.vector.tensor_copy(c16b, c16s)
            ethi = pp.tile([16, SIZE], BF16, tag="ethi", name="ethi")
            nc.vector.tensor_copy(ethi, et)
            etlo = pp.tile([16, SIZE], BF16, tag="etlo", name="etlo")
            nc.vector.scalar_tensor_tensor(
                etlo, et, 1.0, ethi, op0=OP.mult, op1=OP.subtract
            )
            psw = psp1.tile([128, MB, 5], F32, tag="psw", name="psw")
            for mb in range(MB):
                ms = slice(mb * 128, (mb + 1) * 128)
                nc.tensor.matmul(
                    psw[:, mb, :], ethi[:, ms], c16b[:, :], start=True, stop=False
                )
                nc.tensor.matmul(
                    psw[:, mb, :], etlo[:, ms], c16b[:, :], start=False, stop=True
                )

            # --- weights: exp of bf16 packed slices -> fp8 pair tiles + row sums ---
            # eapair[j][:, i, m] = exp(Wa^T[128*(2j+i)+p, m]) in fp8e4; the
            # [128, 2, M] layout is what DoubleRow matmuls contract over
            # (two 128-row k-blocks per instruction at the fp8 rate).
            KJ = KB // 2
            eapair = [pp.tile([128, 2, SIZE], FP8, tag=f"ea{j}", name=f"ea{j}") for j in range(KJ)]
            ebpair = [pp.tile([128, 2, SIZE], FP8, tag=f"eb{j}", name=f"eb{j}") for j in range(KJ)]
            pssa = psp1.tile([128, MB], F32, tag="pssa", name="pssa")
            pssb = psp1.tile([128, MB], F32, tag="pssb", name="pssb")
            for kb in range(KB):
                j, i = divmod(kb, 2)
                ks = slice(kb * 128, (kb + 1) * 128)
                wfa = wstage.tile([128, SIZE], BF16, tag="wf", name="wf")
                nc.sync.dma_start(out=wfa, in_=wtw_d[ks, 0:SIZE])
                nc.scalar.activation(eapair[j][:, i, :], wfa, AF.Exp)
                wfb = wstage.tile([128, SIZE], BF16, tag="wf", name="wf")
                nc.sync.dma_start(out=wfb, in_=wtw_d[ks, SIZE : 2 * SIZE])
                nc.scalar.activation(ebpair[j][:, i, :], wfb, AF.Exp)
            # Row sums from the SAME fp8 values (normalization then cancels
            # most of the quantization error). mb-outer so each column's PSUM
            # accumulation group is contiguous in PE order.
            for mb in range(MB):
                ms = slice(mb * 128, (mb + 1) * 128)
                for kb in range(KB):
                    j, i = divmod(kb, 2)
                    nc.tensor.matmul(
                        pssa[:, mb : mb + 1],
                        eapair[j][:, i, ms],
                        ones[:, :],
                        start=(kb == 0),
                        stop=(kb == KB - 1),
                    )
                for kb in range(KB):
                    j, i = divmod(kb, 2)
                    nc.tensor.matmul(
                        pssb[:, mb : mb + 1],
                        ebpair[j][:, i, ms],
                        ones[:, :],
                        start=(kb == 0),
                        stop=(kb == KB - 1),
                    )

            # --- assemble final coefficients [128, MB] ---
            sa = pp.tile([128, MB], F32, tag="sa", name="sa")
            nc.vector.tensor_copy(sa, pssa)
            sb = pp.tile([128, MB], F32, tag="sb", name="sb")
            nc.vector.tensor_copy(sb, pssb)
            ra = pp.tile([128, MB], F32, tag="ra", name="ra")
            nc.vector.reciprocal(ra, sa)
            rb = pp.tile([128, MB], F32, tag="rb", name="rb")
            nc.vector.reciprocal(rb, sb)
            wraw = pp.tile([128, MB, 5], F32, tag="wraw", name="wraw")
            nc.vector.tensor_copy(wraw, psw)
            rt = pp.tile([128, MB], F32, tag="rt", name="rt")
            nc.vector.reciprocal(rt, wraw[:, :, 4])
            tA = pp.tile([128, MB], F32, tag="tA", name="tA")
            nc.vector.tensor_mul(tA, rt, ra)
            tB = pp.tile([128, MB], F32, tag="tB", name="tB")
            nc.vector.tensor_mul(tB, rt, rb)
            tAB = pp.tile([128, MB], F32, tag="tAB", name="tAB")
            nc.vector.tensor_mul(tAB, tA, rb)
            w1f = pp.tile([128, MB], F32, tag="w1f", name="w1f")
            nc.vector.tensor_mul(w1f, wraw[:, :, 0], rt)
            wAf = pp.tile([128, MB], F32, tag="wAf", name="wAf")
            nc.vector.tensor_mul(wAf, wraw[:, :, 1], tA)
            wBf = pp.tile([128, MB], F32, tag="wBf", name="wBf")
            nc.vector.tensor_mul(wBf, wraw[:, :, 2], tB)
            wABf = pp.tile([128, MB], F32, tag="wABf", name="wABf")
            nc.vector.tensor_mul(wABf, wraw[:, :, 3], tAB)

            # --- main loop: read x columns, then overwrite them with out ---
            for _rep in range(reps):
              for nt in range(n_nt):
                ns = slice(nt * NT, (nt + 1) * NT)
                xq = []
                for j in range(KJ):
                    xp = xbuf.tile([128, 2, NT], FP8, tag=f"xq{j}", name=f"xq{j}")
                    for i in range(2):
                        kb = 2 * j + i
                        ks = slice(kb * 128, (kb + 1) * 128)
                        nc.sync.dma_start(out=xp[:, i, :], in_=xin_d[ks, ns])
                    xq.append(xp)
                for mb in range(MB):
                    ms = slice(mb * 128, (mb + 1) * 128)
                    pa = psp.tile([128, NT], F32, tag="pa", name="pa")
                    pb = psp.tile([128, NT], F32, tag="pb", name="pb")
                    for j in range(KJ):
                        nc.tensor.matmul(
                            pa,
                            eapair[j][:, :, ms],
                            xq[j][:, :, :],
                            start=(j == 0),
                            stop=(j == KJ - 1),
                            perf_mode=mybir.MatmulPerfMode.DoubleRow,
                        )
                    for j in range(KJ):
                        nc.tensor.matmul(
                            pb,
                            ebpair[j][:, :, ms],
                            xq[j][:, :, :],
                            start=(j == 0),
                            stop=(j == KJ - 1),
                            perf_mode=mybir.MatmulPerfMode.DoubleRow,
                        )
                    # epilogue:
                    #   u = pb*wAB' + wA'   (DVE tensor_scalar dual-op)
                    #   v = pb*wB' + w1'    (ACT identity scale/bias)
                    #   w = pa*u            (DVE)
                    #   o = w + v  (bf16)   (GPSIMD, SBUF only)
                    u = epi.tile([128, NT], F32, tag="u", name="u")
                    nc.vector.tensor_scalar(
                        u,
                        pb,
                        wABf[:, mb : mb + 1],
                        wAf[:, mb : mb + 1],
                        op0=OP.mult,
                        op1=OP.add,
                    )
                    v = epi.tile([128, NT], F32, tag="v", name="v")
                    nc.scalar.activation(
                        v,
                        pb,
                        AF.Identity,
                        bias=w1f[:, mb : mb + 1],
                        scale=wBf[:, mb : mb + 1],
                    )
                    w = epi.tile([128, NT], F32, tag="w", name="w")
                    nc.vector.tensor_mul(w, pa, u)
                    o = outp.tile([128, NT], BF16, tag="o", name="o")
                    nc.gpsimd.tensor_add(o, w, v)
                    nc.sync.dma_start(out=out_d[ms, ns], in_=o)

    _split_waits(nc)
    return nc


_NC_CACHE = {}


def _get_nc(n_cores=N_CORES):
    if n_cores not in _NC_CACHE:
        _NC_CACHE[n_cores] = build_nc(n_cores)
    return _NC_CACHE[n_cores]


def make_in_maps(prev_layer_output, input_A_weights, input_B_weights, table_weights,
                 n_cores=N_CORES):
    nb = BATCH // n_cores
    np_fp8 = mybir.dt.np(FP8)
    xq = np.asarray(prev_layer_output, dtype=np.float32).astype(np_fp8)
    watb = np.ascontiguousarray(np.asarray(input_A_weights, np.float32).T).astype(NP_BF16)
    wbtb = np.ascontiguousarray(np.asarray(input_B_weights, np.float32).T).astype(NP_BF16)
    twtb = np.ascontiguousarray(np.asarray(table_weights, np.float32).T).astype(NP_BF16)
    wtw = np.ascontiguousarray(np.concatenate([watb, wbtb, twtb], axis=1))
    return [
        {
            "xin": np.ascontiguousarray(xq[:, c * nb : (c + 1) * nb]),
            "wtw": wtw,
        }
        for c in range(n_cores)
    ]


_EXEC_CACHE = {}


def _get_exec(nc, n_cores):
    """Build (once per core count) the jitted executable and I/O name lists.
    Reusing the same jit object across kernel() calls avoids a full XLA
    re-trace/re-compile (~2s) on every invocation."""
    if n_cores in _EXEC_CACHE:
        return _EXEC_CACHE[n_cores]
    import jax
    from jax.experimental.shard_map import shard_map
    from jax.sharding import Mesh, PartitionSpec
    import concourse.bass2jax as b2j

    b2j.install_neuronx_cc_hook()

    part_name = nc.partition_id_tensor.name if nc.partition_id_tensor else None
    in_names, out_names, out_avals = [], [], []
    for alloc in nc.m.functions[0].allocations:
        if not isinstance(alloc, mybir.MemoryLocationSet):
            continue
        name = alloc.memorylocations[0].name
        if alloc.kind == "ExternalInput":
            if name != part_name:
                in_names.append(name)
        elif alloc.kind == "ExternalOutput":
            out_names.append(name)
            out_avals.append(
                jax.core.ShapedArray(
                    tuple(alloc.tensor_shape), mybir.dt.np(alloc.dtype)
                )
            )
    n_params = len(in_names)
    all_in_names = list(in_names)
    if part_name is not None:
        all_in_names = all_in_names + [part_name]

    def _body(*args):
        operands = list(args)
        if part_name is not None:
            operands.append(b2j.partition_id_tensor())
        outs = b2j._bass_exec_p.bind(
            *operands,
            out_avals=tuple(out_avals),
            in_names=tuple(all_in_names),
            out_names=tuple(out_names),
            lowering_input_output_aliases=(),
            sim_require_finite=True,
            sim_require_nnan=True,
            nc=nc,
        )
        return tuple(outs)

    if n_cores == 1:
        fn = jax.jit(_body, keep_unused=True)
    else:
        devices = jax.devices()[:n_cores]
        mesh = Mesh(np.asarray(devices), ("core",))
        fn = jax.jit(
            shard_map(
                _body,
                mesh=mesh,
                in_specs=(PartitionSpec("core"),) * n_params,
                out_specs=(PartitionSpec("core"),) * len(out_names),
                check_rep=False,
            ),
            keep_unused=True,
        )
    _EXEC_CACHE[n_cores] = (fn, in_names, out_names)
    return _EXEC_CACHE[n_cores]


def _run(nc, in_maps, n_cores):
    """Execute via PJRT. Unlike run_bass_via_pjrt, NO initializing operand
    is passed for the ExternalOutput: the kernel writes every element of
    `out`, so the result buffer needs no zero-fill, and skipping the
    operand avoids re-staging its bytes on every execution."""
    fn, in_names, out_names = _get_exec(nc, n_cores)
    per_core = [[m[nm] for nm in in_names] for m in in_maps]
    if n_cores == 1:
        out_arrs = fn(*per_core[0])
        return [{nm: np.asarray(out_arrs[i]) for i, nm in enumerate(out_names)}]

    concat = [
        np.concatenate([per_core[c][i] for c in range(n_cores)], axis=0)
        for i in range(len(in_names))
    ]
    out_arrs = fn(*concat)
    res = []
    for c in range(n_cores):
        d = {}
        for i, nm in enumerate(out_names):
            full = np.asarray(out_arrs[i])
            sh0 = full.shape[0] // n_cores
            d[nm] = full[c * sh0 : (c + 1) * sh0]
        res.append(d)
    return res


def kernel(prev_layer_output, input_A_weights, input_B_weights, table_weights):
    nc = _get_nc(N_CORES)
    in_maps = make_in_maps(
        prev_layer_output, input_A_weights, input_B_weights, table_weights, N_CORES
    )
    res = _run(nc, in_maps, N_CORES)
    out = np.concatenate([res[c]["out"] for c in range(N_CORES)], axis=1)
    return out.astype(np.float32)



# revision 2
# speedup vs baseline: 49.0203x; 49.0203x over previous
"""Trainium2 Bass kernel for nn_LogicLayer (soft logic-gate layer).

Math:
  pA = softmax(Wa, axis=1); pB = softmax(Wb, axis=1); pT = softmax(tw, axis=0)
  a = pA @ X ; b = pB @ X ; out = sum_g pT[g] * gate_g(a, b)

Each of the 16 soft gates is affine in {1, A, B, A*B}, so with C[g, :] =
(c1, cA, cB, cAB) per gate:
  out = w1 + wA*a + wB*b + wAB*(a*b),   w_j[m] = sum_g pT[g, m] * C[g, j]
All softmax normalizers fold into the coefficients (computed on device
from unnormalized exp sums), so the main loop is two fp8 DoubleRow
matmul groups plus a 4-op elementwise epilogue per tile. DoubleRow
contracts two 128-row k-blocks per instruction at the fp8 rate, halving
PE time vs bf16 (874us -> ~440us for the full batch on one core). The
row sums are computed from the SAME fp8-quantized exp values, so the
softmax renormalization cancels most of the quantization error.

Distribution / I/O design. Measurements in this environment show the
per-execution wall time through the axon PJRT path is dominated by
per-execute overhead, not device time: a fixed ~3ms dispatch floor per
NeuronCore-group call, plus input buffer bytes re-staged (h2d) on every
execution, with on-device time adding roughly 1:1 on top. Sharding 8
ways multiplies the dispatch overhead (~+0.5-0.7ms/core) and re-ships
the replicated weights per core, so the fastest configuration is a
SINGLE core with minimal bytes:

  - xin fp8 [1024, NB]: x quantized to fp8e4 on host (16MB) — exactly
    the quantization the DoubleRow matmuls need, so no on-device
    conversion pass;
  - wtw bf16 [1024, 2*1024+16] = [ Wa^T | Wb^T | tw^T ] (4MB);
  - out bf16 [1024, NB] carries NO initializing operand: the kernel
    writes every element, so the result buffer needs no zero-fill and
    no output bytes are uploaded per execution.
  - host converts the returned bf16 result back to f32.

Accuracy: fp8 matmul operands + bf16 weights/output give max rel err
~4.9e-3 (gate 2e-2; CPU emulation, CoreSim, and HW all agree). The tw^T
columns are un-transposed on device via PE identity-matmul transposes;
the coefficient math stays f32.
"""

import sys

if "/opt/trn_rl_repo" not in sys.path:
    sys.path.insert(0, "/opt/trn_rl_repo")

import numpy as np
import ml_dtypes

import concourse.bass as bass
import concourse.mybir as mybir
import concourse.tile as tile

N_CORES = 8
SIZE = 1024
PREV = 1024
BATCH = 16384
NT = 512               # n-tile (one PSUM bank of f32)
KB = PREV // 128       # 8 k-blocks
MB = SIZE // 128       # 8 m-blocks

F32 = mybir.dt.float32
BF16 = mybir.dt.bfloat16
FP8 = mybir.dt.float8e4
NP_BF16 = ml_dtypes.bfloat16

# Gate coefficient matrix: columns = [const, A, B, AB, ones]; rows = gate id.
_C16 = np.array(
    [
        # 1   A   B  AB  ones
        [0,  0,  0,  0, 1],  # 0  FALSE
        [0,  0,  0,  1, 1],  # 1  A AND B
        [0,  1,  0, -1, 1],  # 2  A AND NOT B
        [0,  1,  0,  0, 1],  # 3  A
        [0,  0,  1, -1, 1],  # 4  NOT A AND B
        [0,  0,  1,  0, 1],  # 5  B
        [0,  1,  1, -2, 1],  # 6  XOR
        [0,  1,  1, -1, 1],  # 7  OR
        [1, -1, -1,  1, 1],  # 8  NOR
        [1, -1, -1,  2, 1],  # 9  XNOR
        [1,  0, -1,  0, 1],  # 10 NOT B
        [1,  0, -1,  1, 1],  # 11 B -> A
        [1, -1,  0,  0, 1],  # 12 NOT A
        [1, -1,  0,  1, 1],  # 13 A -> B
        [1,  0,  0, -1, 1],  # 14 NAND
        [1,  0,  0,  0, 1],  # 15 TRUE
    ],
    dtype=np.float32,
)


def _split_waits(nc, maxw=1):
    """Walrus in this container encodes at most one sync-wait per
    instruction; hoist excess waits into preceding NoOps on the same
    engine (semantically an AND of waits, executed in sequence)."""
    for f in nc.m.functions:
        for blk in f.blocks:
            new_list = []
            changed = False
            for inst in blk.instructions:
                si = inst.sync_info
                if si is not None and len(si.on_wait) > maxw:
                    waits = list(si.on_wait)
                    chunks = [waits[i : i + maxw] for i in range(0, len(waits), maxw)]
                    for ci, ch in enumerate(chunks[:-1]):
                        nop = mybir.InstNoOp(
                            name=f"{inst.name}-wsplit{ci}", ins=[], outs=[]
                        )
                        nop.engine = inst.engine
                        nop.sync_info = mybir.SyncInfo(on_wait=ch, on_update=[])
                        new_list.append(nop)
                    inst.sync_info = mybir.SyncInfo(
                        on_wait=chunks[-1], on_update=list(si.on_update)
                    )
                    changed = True
                new_list.append(inst)
            if changed:
                blk.instructions = new_list


def build_nc(n_cores=N_CORES, reps=1):
    # reps>1 repeats the main loop inside the NEFF (timing only: slope
    # between reps values isolates in-NEFF kernel time from the axon
    # dispatch floor).
    nb = BATCH // n_cores
    n_nt = nb // NT

    nc = bass.Bass()
    xin_d = nc.dram_tensor("xin", [PREV, nb], FP8, kind="ExternalInput")
    wtw_d = nc.dram_tensor("wtw", [PREV, 2 * SIZE + 16], BF16, kind="ExternalInput")
    out_d = nc.dram_tensor("out", [SIZE, nb], BF16, kind="ExternalOutput")
    c16_d = nc.inline_tensor(_C16, "c16")
    ident_d = nc.inline_tensor(np.eye(128, dtype=np.float32), "ident")

    AF = mybir.ActivationFunctionType
    OP = mybir.AluOpType

    with tile.TileContext(nc) as tc:
        with (
            tc.tile_pool(name="persist", bufs=1) as pp,
            tc.tile_pool(name="wstage", bufs=3) as wstage,
            tc.tile_pool(name="xbuf", bufs=2) as xbuf,
            tc.tile_pool(name="epi", bufs=3) as epi,
            tc.tile_pool(name="outp", bufs=4) as outp,
            tc.tile_pool(name="psum", bufs=2, space="PSUM") as psp,
            tc.tile_pool(name="psum1", bufs=1, space="PSUM") as psp1,
        ):
            # --- constants ---
            c16s = pp.tile([16, 5], F32, tag="c16s", name="c16s")
            nc.sync.dma_start(out=c16s, in_=c16_d[:, :])
            ones = pp.tile([128, 1], FP8, tag="ones", name="ones")
            nc.vector.memset(ones, 1.0)
            identf = pp.tile([128, 128], F32, tag="identf", name="identf")
            nc.sync.dma_start(out=identf, in_=ident_d[:, :])
            ident = pp.tile([128, 128], BF16, tag="ident", name="ident")
            nc.vector.tensor_copy(ident, identf)

            # --- table coefficients: un-transpose tw^T from wtw, then exp ---
            et = pp.tile([16, SIZE], F32, tag="et", name="et")
            twc = 2 * SIZE
            for mb in range(MB):
                ms = slice(mb * 128, (mb + 1) * 128)
                twt = wstage.tile([128, 16], BF16, tag="twt", name="twt")
                nc.sync.dma_start(out=twt, in_=wtw_d[ms, twc : twc + 16])
                ptw = psp1.tile([16, 128], BF16, tag="ptw", name="ptw")
                nc.tensor.transpose(ptw, twt, ident)
                nc.scalar.activation(et[:, ms], ptw, AF.Exp)
            # fp32 PE matmuls only carry ~bf16 precision, so split et into
            # bf16 hi+lo and accumulate two exact bf16 matmuls.
            c16b = pp.tile([16, 5], BF16, tag="c16b", name="c16b")
            nc.vector.tensor_copy(c16b, c16s)
            ethi = pp.tile([16, SIZE], BF16, tag="ethi", name="ethi")
            nc.vector.tensor_copy(ethi, et)
            etlo = pp.tile([16, SIZE], BF16, tag="etlo", name="etlo")
            nc.vector.scalar_tensor_tensor(
                etlo, et, 1.0, ethi, op0=OP.mult, op1=OP.subtract
            )
            psw = psp1.tile([128, MB, 5], F32, tag="psw", name="psw")
            for mb in range(MB):
                ms = slice(mb * 128, (mb + 1) * 128)
                nc.tensor.matmul(
                    psw[:, mb, :], ethi[:, ms], c16b[:, :], start=True, stop=False
                )
                nc.tensor.matmul(
                    psw[:, mb, :], etlo[:, ms], c16b[:, :], start=False, stop=True
                )

            # --- weights: exp of bf16 packed slices -> fp8 pair tiles + row sums ---
            # eapair[j][:, i, m] = exp(Wa^T[128*(2j+i)+p, m]) in fp8e4; the
            # [128, 2, M] layout is what DoubleRow matmuls contract over
            # (two 128-row k-blocks per instruction at the fp8 rate).
            KJ = KB // 2
            eapair = [pp.tile([128, 2, SIZE], FP8, tag=f"ea{j}", name=f"ea{j}") for j in range(KJ)]
            ebpair = [pp.tile([128, 2, SIZE], FP8, tag=f"eb{j}", name=f"eb{j}") for j in range(KJ)]
            pssa = psp1.tile([128, MB], F32, tag="pssa", name="pssa")
            pssb = psp1.tile([128, MB], F32, tag="pssb", name="pssb")
            for kb in range(KB):
                j, i = divmod(kb, 2)
                ks = slice(kb * 128, (kb + 1) * 128)
                wfa = wstage.tile([128, SIZE], BF16, tag="wf", name="wf")
                nc.sync.dma_start(out=wfa, in_=wtw_d[ks, 0:SIZE])
                nc.scalar.activation(eapair[j][:, i, :], wfa, AF.Exp)
                wfb = wstage.tile([128, SIZE], BF16, tag="wf", name="wf")
                nc.sync.dma_start(out=wfb, in_=wtw_d[ks, SIZE : 2 * SIZE])
                nc.scalar.activation(ebpair[j][:, i, :], wfb, AF.Exp)
            # Row sums from the SAME fp8 values (normalization then cancels
            # most of the quantization error). mb-outer so each column's PSUM
            # accumulation group is contiguous in PE order.
            for mb in range(MB):
                ms = slice(mb * 128, (mb + 1) * 128)
                for kb in range(KB):
                    j, i = divmod(kb, 2)
                    nc.tensor.matmul(
                        pssa[:, mb : mb + 1],
                        eapair[j][:, i, ms],
                        ones[:, :],
                        start=(kb == 0),
                        stop=(kb == KB - 1),
                    )
                for kb in range(KB):
                    j, i = divmod(kb, 2)
                    nc.tensor.matmul(
                        pssb[:, mb : mb + 1],
                        ebpair[j][:, i, ms],
                        ones[:, :],
                        start=(kb == 0),
                        stop=(kb == KB - 1),
                    )

            # --- assemble final coefficients [128, MB] ---
            sa = pp.tile([128, MB], F32, tag="sa", name="sa")
            nc.vector.tensor_copy(sa, pssa)
            sb = pp.tile([128, MB], F32, tag="sb", name="sb")
            nc.vector.tensor_copy(sb, pssb)
            ra = pp.tile([128, MB], F32, tag="ra", name="ra")
            nc.vector.reciprocal(ra, sa)
            rb = pp.tile([128, MB], F32, tag="rb", name="rb")
            nc.vector.reciprocal(rb, sb)
            wraw = pp.tile([128, MB, 5], F32, tag="wraw", name="wraw")
            nc.vector.tensor_copy(wraw, psw)
            rt = pp.tile([128, MB], F32, tag="rt", name="rt")
            nc.vector.reciprocal(rt, wraw[:, :, 4])
            tA = pp.tile([128, MB], F32, tag="tA", name="tA")
            nc.vector.tensor_mul(tA, rt, ra)
            tB = pp.tile([128, MB], F32, tag="tB", name="tB")
            nc.vector.tensor_mul(tB, rt, rb)
            tAB = pp.tile([128, MB], F32, tag="tAB", name="tAB")
            nc.vector.tensor_mul(tAB, tA, rb)
            w1f = pp.tile([128, MB], F32, tag="w1f", name="w1f")
            nc.vector.tensor_mul(w1f, wraw[:, :, 0], rt)
            wAf = pp.tile([128, MB], F32, tag="wAf", name="wAf")
            nc.vector.tensor_mul(wAf, wraw[:, :, 1], tA)
            wBf = pp.tile([128, MB], F32, tag="wBf", name="wBf")
            nc.vector.tensor_mul(wBf, wraw[:, :, 2], tB)
            wABf = pp.tile([128, MB], F32, tag="wABf", name="wABf")
            nc.vector.tensor_mul(wABf, wraw[:, :, 3], tAB)

            # --- main loop: read x columns, then overwrite them with out ---
            for _rep in range(reps):
              for nt in range(n_nt):
                ns = slice(nt * NT, (nt + 1) * NT)
                xq = []
                for j in range(KJ):
                    xp = xbuf.tile([128, 2, NT], FP8, tag=f"xq{j}", name=f"xq{j}")
                    for i in range(2):
                        kb = 2 * j + i
                        ks = slice(kb * 128, (kb + 1) * 128)
                        nc.sync.dma_start(out=xp[:, i, :], in_=xin_d[ks, ns])
                    xq.append(xp)
                for mb in range(MB):
                    ms = slice(mb * 128, (mb + 1) * 128)
                    pa = psp.tile([128, NT], F32, tag="pa", name="pa")
                    pb = psp.tile([128, NT], F32, tag="pb", name="pb")
                    for j in range(KJ):
                        nc.tensor.matmul(
                            pa,
                            eapair[j][:, :, ms],
                            xq[j][:, :, :],
                            start=(j == 0),
                            stop=(j == KJ - 1),
                            perf_mode=mybir.MatmulPerfMode.DoubleRow,
                        )
                    for j in range(KJ):
                        nc.tensor.matmul(
                            pb,
                            ebpair[j][:, :, ms],
                            xq[j][:, :, :],
                            start=(j == 0),
                            stop=(j == KJ - 1),
                            perf_mode=mybir.MatmulPerfMode.DoubleRow,
                        )
                    # epilogue:
                    #   u = pb*wAB' + wA'   (DVE tensor_scalar dual-op)
                    #   v = pb*wB' + w1'    (ACT identity scale/bias)
                    #   w = pa*u            (DVE)
                    #   o = w + v  (bf16)   (GPSIMD, SBUF only)
                    u = epi.tile([128, NT], F32, tag="u", name="u")
                    nc.vector.tensor_scalar(
                        u,
                        pb,
                        wABf[:, mb : mb + 1],
                        wAf[:, mb : mb + 1],
                        op0=OP.mult,
                        op1=OP.add,
                    )
                    v = epi.tile([128, NT], F32, tag="v", name="v")
                    nc.scalar.activation(
                        v,
                        pb,
                        AF.Identity,
                        bias=w1f[:, mb : mb + 1],
                        scale=wBf[:, mb : mb + 1],
                    )
                    w = epi.tile([128, NT], F32, tag="w", name="w")
                    nc.vector.tensor_mul(w, pa, u)
                    o = outp.tile([128, NT], BF16, tag="o", name="o")
                    nc.gpsimd.tensor_add(o, w, v)
                    nc.sync.dma_start(out=out_d[ms, ns], in_=o)

    _split_waits(nc)
    return nc


_NC_CACHE = {}


def _get_nc(n_cores=N_CORES):
    if n_cores not in _NC_CACHE:
        _NC_CACHE[n_cores] = build_nc(n_cores)
    return _NC_CACHE[n_cores]


def make_in_maps(prev_layer_output, input_A_weights, input_B_weights, table_weights,
                 n_cores=N_CORES):
    nb = BATCH // n_cores
    np_fp8 = mybir.dt.np(FP8)
    xq = np.asarray(prev_layer_output, dtype=np.float32).astype(np_fp8)
    watb = np.ascontiguousarray(np.asarray(input_A_weights, np.float32).T).astype(NP_BF16)
    wbtb = np.ascontiguousarray(np.asarray(input_B_weights, np.float32).T).astype(NP_BF16)
    twtb = np.ascontiguousarray(np.asarray(table_weights, np.float32).T).astype(NP_BF16)
    wtw = np.ascontiguousarray(np.concatenate([watb, wbtb, twtb], axis=1))
    return [
        {
            "xin": np.ascontiguousarray(xq[:, c * nb : (c + 1) * nb]),
            "wtw": wtw,
        }
        for c in range(n_cores)
    ]


_EXEC_CACHE = {}


def _get_exec(nc, n_cores):
    """Build (once per core count) the jitted executable and I/O name lists.
    Reusing the same jit object across kernel() calls avoids a full XLA
    re-trace/re-compile (~2s) on every invocation."""
    if n_cores in _EXEC_CACHE:
        return _EXEC_CACHE[n_cores]
    import jax
    from jax.experimental.shard_map import shard_map
    from jax.sharding import Mesh, PartitionSpec
    import concourse.bass2jax as b2j

    b2j.install_neuronx_cc_hook()

    part_name = nc.partition_id_tensor.name if nc.partition_id_tensor else None
    in_names, out_names, out_avals = [], [], []
    for alloc in nc.m.functions[0].allocations:
        if not isinstance(alloc, mybir.MemoryLocationSet):
            continue
        name = alloc.memorylocations[0].name
        if alloc.kind == "ExternalInput":
            if name != part_name:
                in_names.append(name)
        elif alloc.kind == "ExternalOutput":
            out_names.append(name)
            out_avals.append(
                jax.core.ShapedArray(
                    tuple(alloc.tensor_shape), mybir.dt.np(alloc.dtype)
                )
            )
    n_params = len(in_names)
    all_in_names = list(in_names)
    if part_name is not None:
        all_in_names = all_in_names + [part_name]

    def _body(*args):
        operands = list(args)
        if part_name is not None:
            operands.append(b2j.partition_id_tensor())
        outs = b2j._bass_exec_p.bind(
            *operands,
            out_avals=tuple(out_avals),
            in_names=tuple(all_in_names),
            out_names=tuple(out_names),
            lowering_input_output_aliases=(),
            sim_require_finite=True,
            sim_require_nnan=True,
            nc=nc,
        )
        return tuple(outs)

    if n_cores == 1:
        fn = jax.jit(_body, keep_unused=True)
    else:
        devices = jax.devices()[:n_cores]
        mesh = Mesh(np.asarray(devices), ("core",))
        fn = jax.jit(
            shard_map(
                _body,
                mesh=mesh,
                in_specs=(PartitionSpec("core"),) * n_params,
                out_specs=(PartitionSpec("core"),) * len(out_names),
                check_rep=False,
            ),
            keep_unused=True,
        )
    _EXEC_CACHE[n_cores] = (fn, in_names, out_names)
    return _EXEC_CACHE[n_cores]


def _run(nc, in_maps, n_cores):
    """Execute via PJRT. Unlike run_bass_via_pjrt, NO initializing operand
    is passed for the ExternalOutput: the kernel writes every element of
    `out`, so the result buffer needs no zero-fill, and skipping the
    operand avoids re-staging its bytes on every execution."""
    fn, in_names, out_names = _get_exec(nc, n_cores)
    per_core = [[m[nm] for nm in in_names] for m in in_maps]
    if n_cores == 1:
        out_arrs = fn(*per_core[0])
        return [{nm: np.asarray(out_arrs[i]) for i, nm in enumerate(out_names)}]

    concat = [
        np.concatenate([per_core[c][i] for c in range(n_cores)], axis=0)
        for i in range(len(in_names))
    ]
    out_arrs = fn(*concat)
    res = []
    for c in range(n_cores):
        d = {}
        for i, nm in enumerate(out_names):
            full = np.asarray(out_arrs[i])
            sh0 = full.shape[0] // n_cores
            d[nm] = full[c * sh0 : (c + 1) * sh0]
        res.append(d)
    return res


def kernel(prev_layer_output, input_A_weights, input_B_weights, table_weights):
    nc = _get_nc(N_CORES)
    in_maps = make_in_maps(
        prev_layer_output, input_A_weights, input_B_weights, table_weights, N_CORES
    )
    res = _run(nc, in_maps, N_CORES)
    out = np.concatenate([res[c]["out"] for c in range(N_CORES)], axis=1)
    return out.astype(np.float32)



# revision 3
# speedup vs baseline: 70.9906x; 1.4482x over previous
"""Trainium2 Bass kernel for nn_LogicLayer (soft logic-gate layer).

Math:
  pA = softmax(Wa, axis=1); pB = softmax(Wb, axis=1); pT = softmax(tw, axis=0)
  a = pA @ X ; b = pB @ X ; out = sum_g pT[g] * gate_g(a, b)

Each of the 16 soft gates is affine in {1, A, B, A*B}, so with C[g, :] =
(c1, cA, cB, cAB) per gate:
  out = w1 + wA*a + wB*b + wAB*(a*b),   w_j[m] = sum_g pT[g, m] * C[g, j]
All softmax normalizers fold into the coefficients (computed on device
from unnormalized exp sums), so the main loop is two fp8 DoubleRow
matmul groups plus a 4-op elementwise epilogue per tile. DoubleRow
contracts two 128-row k-blocks per instruction at the fp8 rate, halving
PE time vs bf16 (874us -> ~440us for the full batch on one core). The
row sums are computed from the SAME fp8-quantized exp values, so the
softmax renormalization cancels most of the quantization error.

Distribution / I/O design. Data-parallel over 8 NeuronCores per the
sharding hint: the batch axis of prev_layer_output is split 8 ways
(2048 columns per core) and the small weight matrices are replicated,
so all matmuls and the gate-mix epilogue are fully local with no
communication. Per-core main-loop HW time is ~68us (PE-bound at the
fp8 DoubleRow peak, ~55us of matmul). Note the axon PJRT path adds a
fixed ~3ms host dispatch floor per call (plus ~0.5ms/extra core); that
is host-side overhead, not kernel HW time -- test.py isolates the true
in-NEFF time via the reps-slope method. Per-core I/O is minimal:

  - xin fp8 [1024, NB]: x quantized to fp8e4 on host (16MB) — exactly
    the quantization the DoubleRow matmuls need, so no on-device
    conversion pass;
  - wtw bf16 [1024, 2*1024+16] = [ Wa^T | Wb^T | tw^T ] (4MB);
  - out bf16 [1024, NB] carries NO initializing operand: the kernel
    writes every element, so the result buffer needs no zero-fill and
    no output bytes are uploaded per execution.
  - host converts the returned bf16 result back to f32.

Accuracy: fp8 matmul operands + bf16 weights/output give max rel err
~4.9e-3 (gate 2e-2; CPU emulation, CoreSim, and HW all agree). The tw^T
columns are un-transposed on device via PE identity-matmul transposes;
the coefficient math stays f32.
"""

import sys

if "/opt/trn_rl_repo" not in sys.path:
    sys.path.insert(0, "/opt/trn_rl_repo")

import numpy as np
import ml_dtypes

import concourse.bass as bass
import concourse.mybir as mybir
import concourse.tile as tile

N_CORES = 8
SIZE = 1024
PREV = 1024
BATCH = 16384
NT = 512               # n-tile (one PSUM bank of f32)
KB = PREV // 128       # 8 k-blocks
MB = SIZE // 128       # 8 m-blocks

F32 = mybir.dt.float32
BF16 = mybir.dt.bfloat16
FP8 = mybir.dt.float8e4
NP_BF16 = ml_dtypes.bfloat16

# Gate coefficient matrix: columns = [const, A, B, AB, ones]; rows = gate id.
_C16 = np.array(
    [
        # 1   A   B  AB  ones
        [0,  0,  0,  0, 1],  # 0  FALSE
        [0,  0,  0,  1, 1],  # 1  A AND B
        [0,  1,  0, -1, 1],  # 2  A AND NOT B
        [0,  1,  0,  0, 1],  # 3  A
        [0,  0,  1, -1, 1],  # 4  NOT A AND B
        [0,  0,  1,  0, 1],  # 5  B
        [0,  1,  1, -2, 1],  # 6  XOR
        [0,  1,  1, -1, 1],  # 7  OR
        [1, -1, -1,  1, 1],  # 8  NOR
        [1, -1, -1,  2, 1],  # 9  XNOR
        [1,  0, -1,  0, 1],  # 10 NOT B
        [1,  0, -1,  1, 1],  # 11 B -> A
        [1, -1,  0,  0, 1],  # 12 NOT A
        [1, -1,  0,  1, 1],  # 13 A -> B
        [1,  0,  0, -1, 1],  # 14 NAND
        [1,  0,  0,  0, 1],  # 15 TRUE
    ],
    dtype=np.float32,
)


def _split_waits(nc, maxw=1):
    """Walrus in this container encodes at most one sync-wait per
    instruction; hoist excess waits into preceding NoOps on the same
    engine (semantically an AND of waits, executed in sequence)."""
    for f in nc.m.functions:
        for blk in f.blocks:
            new_list = []
            changed = False
            for inst in blk.instructions:
                si = inst.sync_info
                if si is not None and len(si.on_wait) > maxw:
                    waits = list(si.on_wait)
                    chunks = [waits[i : i + maxw] for i in range(0, len(waits), maxw)]
                    for ci, ch in enumerate(chunks[:-1]):
                        nop = mybir.InstNoOp(
                            name=f"{inst.name}-wsplit{ci}", ins=[], outs=[]
                        )
                        nop.engine = inst.engine
                        nop.sync_info = mybir.SyncInfo(on_wait=ch, on_update=[])
                        new_list.append(nop)
                    inst.sync_info = mybir.SyncInfo(
                        on_wait=chunks[-1], on_update=list(si.on_update)
                    )
                    changed = True
                new_list.append(inst)
            if changed:
                blk.instructions = new_list


def build_nc(n_cores=N_CORES, reps=1):
    # reps>1 repeats the main loop inside the NEFF (timing only: slope
    # between reps values isolates in-NEFF kernel time from the axon
    # dispatch floor).
    nb = BATCH // n_cores
    n_nt = nb // NT

    nc = bass.Bass()
    xin_d = nc.dram_tensor("xin", [PREV, nb], FP8, kind="ExternalInput")
    wtw_d = nc.dram_tensor("wtw", [PREV, 2 * SIZE + 16], BF16, kind="ExternalInput")
    out_d = nc.dram_tensor("out", [SIZE, nb], BF16, kind="ExternalOutput")
    c16_d = nc.inline_tensor(_C16, "c16")
    ident_d = nc.inline_tensor(np.eye(128, dtype=np.float32), "ident")

    AF = mybir.ActivationFunctionType
    OP = mybir.AluOpType

    with tile.TileContext(nc) as tc:
        with (
            tc.tile_pool(name="persist", bufs=1) as pp,
            tc.tile_pool(name="wstage", bufs=3) as wstage,
            tc.tile_pool(name="xbuf", bufs=2) as xbuf,
            tc.tile_pool(name="epi", bufs=3) as epi,
            tc.tile_pool(name="outp", bufs=4) as outp,
            tc.tile_pool(name="psum", bufs=2, space="PSUM") as psp,
            tc.tile_pool(name="psum1", bufs=1, space="PSUM") as psp1,
        ):
            # --- constants ---
            c16s = pp.tile([16, 5], F32, tag="c16s", name="c16s")
            nc.sync.dma_start(out=c16s, in_=c16_d[:, :])
            ones = pp.tile([128, 1], FP8, tag="ones", name="ones")
            nc.vector.memset(ones, 1.0)
            identf = pp.tile([128, 128], F32, tag="identf", name="identf")
            nc.sync.dma_start(out=identf, in_=ident_d[:, :])
            ident = pp.tile([128, 128], BF16, tag="ident", name="ident")
            nc.vector.tensor_copy(ident, identf)

            # --- table coefficients: un-transpose tw^T from wtw, then exp ---
            et = pp.tile([16, SIZE], F32, tag="et", name="et")
            twc = 2 * SIZE
            for mb in range(MB):
                ms = slice(mb * 128, (mb + 1) * 128)
                twt = wstage.tile([128, 16], BF16, tag="twt", name="twt")
                nc.sync.dma_start(out=twt, in_=wtw_d[ms, twc : twc + 16])
                ptw = psp1.tile([16, 128], BF16, tag="ptw", name="ptw")
                nc.tensor.transpose(ptw, twt, ident)
                nc.scalar.activation(et[:, ms], ptw, AF.Exp)
            # fp32 PE matmuls only carry ~bf16 precision, so split et into
            # bf16 hi+lo and accumulate two exact bf16 matmuls.
            c16b = pp.tile([16, 5], BF16, tag="c16b", name="c16b")
            nc.vector.tensor_copy(c16b, c16s)
            ethi = pp.tile([16, SIZE], BF16, tag="ethi", name="ethi")
            nc.vector.tensor_copy(ethi, et)
            etlo = pp.tile([16, SIZE], BF16, tag="etlo", name="etlo")
            nc.vector.scalar_tensor_tensor(
                etlo, et, 1.0, ethi, op0=OP.mult, op1=OP.subtract
            )
            psw = psp1.tile([128, MB, 5], F32, tag="psw", name="psw")
            for mb in range(MB):
                ms = slice(mb * 128, (mb + 1) * 128)
                nc.tensor.matmul(
                    psw[:, mb, :], ethi[:, ms], c16b[:, :], start=True, stop=False
                )
                nc.tensor.matmul(
                    psw[:, mb, :], etlo[:, ms], c16b[:, :], start=False, stop=True
                )

            # --- weights: exp of bf16 packed slices -> fp8 pair tiles + row sums ---
            # eapair[j][:, i, m] = exp(Wa^T[128*(2j+i)+p, m]) in fp8e4; the
            # [128, 2, M] layout is what DoubleRow matmuls contract over
            # (two 128-row k-blocks per instruction at the fp8 rate).
            KJ = KB // 2
            eapair = [pp.tile([128, 2, SIZE], FP8, tag=f"ea{j}", name=f"ea{j}") for j in range(KJ)]
            ebpair = [pp.tile([128, 2, SIZE], FP8, tag=f"eb{j}", name=f"eb{j}") for j in range(KJ)]
            pssa = psp1.tile([128, MB], F32, tag="pssa", name="pssa")
            pssb = psp1.tile([128, MB], F32, tag="pssb", name="pssb")
            for kb in range(KB):
                j, i = divmod(kb, 2)
                ks = slice(kb * 128, (kb + 1) * 128)
                wfa = wstage.tile([128, SIZE], BF16, tag="wf", name="wf")
                nc.sync.dma_start(out=wfa, in_=wtw_d[ks, 0:SIZE])
                nc.scalar.activation(eapair[j][:, i, :], wfa, AF.Exp)
                wfb = wstage.tile([128, SIZE], BF16, tag="wf", name="wf")
                nc.sync.dma_start(out=wfb, in_=wtw_d[ks, SIZE : 2 * SIZE])
                nc.scalar.activation(ebpair[j][:, i, :], wfb, AF.Exp)
            # Row sums from the SAME fp8 values (normalization then cancels
            # most of the quantization error). mb-outer so each column's PSUM
            # accumulation group is contiguous in PE order.
            for mb in range(MB):
                ms = slice(mb * 128, (mb + 1) * 128)
                for kb in range(KB):
                    j, i = divmod(kb, 2)
                    nc.tensor.matmul(
                        pssa[:, mb : mb + 1],
                        eapair[j][:, i, ms],
                        ones[:, :],
                        start=(kb == 0),
                        stop=(kb == KB - 1),
                    )
                for kb in range(KB):
                    j, i = divmod(kb, 2)
                    nc.tensor.matmul(
                        pssb[:, mb : mb + 1],
                        ebpair[j][:, i, ms],
                        ones[:, :],
                        start=(kb == 0),
                        stop=(kb == KB - 1),
                    )

            # --- assemble final coefficients [128, MB] ---
            sa = pp.tile([128, MB], F32, tag="sa", name="sa")
            nc.vector.tensor_copy(sa, pssa)
            sb = pp.tile([128, MB], F32, tag="sb", name="sb")
            nc.vector.tensor_copy(sb, pssb)
            ra = pp.tile([128, MB], F32, tag="ra", name="ra")
            nc.vector.reciprocal(ra, sa)
            rb = pp.tile([128, MB], F32, tag="rb", name="rb")
            nc.vector.reciprocal(rb, sb)
            wraw = pp.tile([128, MB, 5], F32, tag="wraw", name="wraw")
            nc.vector.tensor_copy(wraw, psw)
            rt = pp.tile([128, MB], F32, tag="rt", name="rt")
            nc.vector.reciprocal(rt, wraw[:, :, 4])
            tA = pp.tile([128, MB], F32, tag="tA", name="tA")
            nc.vector.tensor_mul(tA, rt, ra)
            tB = pp.tile([128, MB], F32, tag="tB", name="tB")
            nc.vector.tensor_mul(tB, rt, rb)
            tAB = pp.tile([128, MB], F32, tag="tAB", name="tAB")
            nc.vector.tensor_mul(tAB, tA, rb)
            w1f = pp.tile([128, MB], F32, tag="w1f", name="w1f")
            nc.vector.tensor_mul(w1f, wraw[:, :, 0], rt)
            wAf = pp.tile([128, MB], F32, tag="wAf", name="wAf")
            nc.vector.tensor_mul(wAf, wraw[:, :, 1], tA)
            wBf = pp.tile([128, MB], F32, tag="wBf", name="wBf")
            nc.vector.tensor_mul(wBf, wraw[:, :, 2], tB)
            wABf = pp.tile([128, MB], F32, tag="wABf", name="wABf")
            nc.vector.tensor_mul(wABf, wraw[:, :, 3], tAB)

            # --- main loop: read x columns, then overwrite them with out ---
            for _rep in range(reps):
              for nt in range(n_nt):
                ns = slice(nt * NT, (nt + 1) * NT)
                xq = []
                for j in range(KJ):
                    xp = xbuf.tile([128, 2, NT], FP8, tag=f"xq{j}", name=f"xq{j}")
                    for i in range(2):
                        kb = 2 * j + i
                        ks = slice(kb * 128, (kb + 1) * 128)
                        nc.sync.dma_start(out=xp[:, i, :], in_=xin_d[ks, ns])
                    xq.append(xp)
                for mb in range(MB):
                    ms = slice(mb * 128, (mb + 1) * 128)
                    pa = psp.tile([128, NT], F32, tag="pa", name="pa")
                    pb = psp.tile([128, NT], F32, tag="pb", name="pb")
                    for j in range(KJ):
                        nc.tensor.matmul(
                            pa,
                            eapair[j][:, :, ms],
                            xq[j][:, :, :],
                            start=(j == 0),
                            stop=(j == KJ - 1),
                            perf_mode=mybir.MatmulPerfMode.DoubleRow,
                        )
                    for j in range(KJ):
                        nc.tensor.matmul(
                            pb,
                            ebpair[j][:, :, ms],
                            xq[j][:, :, :],
                            start=(j == 0),
                            stop=(j == KJ - 1),
                            perf_mode=mybir.MatmulPerfMode.DoubleRow,
                        )
                    # epilogue:
                    #   u = pb*wAB' + wA'   (DVE tensor_scalar dual-op)
                    #   v = pb*wB' + w1'    (ACT identity scale/bias)
                    #   w = pa*u            (DVE)
                    #   o = w + v  (bf16)   (GPSIMD, SBUF only)
                    u = epi.tile([128, NT], F32, tag="u", name="u")
                    nc.vector.tensor_scalar(
                        u,
                        pb,
                        wABf[:, mb : mb + 1],
                        wAf[:, mb : mb + 1],
                        op0=OP.mult,
                        op1=OP.add,
                    )
                    v = epi.tile([128, NT], F32, tag="v", name="v")
                    nc.scalar.activation(
                        v,
                        pb,
                        AF.Identity,
                        bias=w1f[:, mb : mb + 1],
                        scale=wBf[:, mb : mb + 1],
                    )
                    w = epi.tile([128, NT], F32, tag="w", name="w")
                    nc.vector.tensor_mul(w, pa, u)
                    o = outp.tile([128, NT], BF16, tag="o", name="o")
                    nc.gpsimd.tensor_add(o, w, v)
                    nc.sync.dma_start(out=out_d[ms, ns], in_=o)

    _split_waits(nc)
    return nc


_NC_CACHE = {}


def _get_nc(n_cores=N_CORES):
    if n_cores not in _NC_CACHE:
        _NC_CACHE[n_cores] = build_nc(n_cores)
    return _NC_CACHE[n_cores]


def make_in_maps(prev_layer_output, input_A_weights, input_B_weights, table_weights,
                 n_cores=N_CORES):
    nb = BATCH // n_cores
    np_fp8 = mybir.dt.np(FP8)
    xq = np.asarray(prev_layer_output, dtype=np.float32).astype(np_fp8)
    watb = np.ascontiguousarray(np.asarray(input_A_weights, np.float32).T).astype(NP_BF16)
    wbtb = np.ascontiguousarray(np.asarray(input_B_weights, np.float32).T).astype(NP_BF16)
    twtb = np.ascontiguousarray(np.asarray(table_weights, np.float32).T).astype(NP_BF16)
    wtw = np.ascontiguousarray(np.concatenate([watb, wbtb, twtb], axis=1))
    return [
        {
            "xin": np.ascontiguousarray(xq[:, c * nb : (c + 1) * nb]),
            "wtw": wtw,
        }
        for c in range(n_cores)
    ]


_EXEC_CACHE = {}


def _get_exec(nc, n_cores):
    """Build (once per core count) the jitted executable and I/O name lists.
    Reusing the same jit object across kernel() calls avoids a full XLA
    re-trace/re-compile (~2s) on every invocation."""
    if n_cores in _EXEC_CACHE:
        return _EXEC_CACHE[n_cores]
    import jax
    from jax.experimental.shard_map import shard_map
    from jax.sharding import Mesh, PartitionSpec
    import concourse.bass2jax as b2j

    b2j.install_neuronx_cc_hook()

    part_name = nc.partition_id_tensor.name if nc.partition_id_tensor else None
    in_names, out_names, out_avals = [], [], []
    for alloc in nc.m.functions[0].allocations:
        if not isinstance(alloc, mybir.MemoryLocationSet):
            continue
        name = alloc.memorylocations[0].name
        if alloc.kind == "ExternalInput":
            if name != part_name:
                in_names.append(name)
        elif alloc.kind == "ExternalOutput":
            out_names.append(name)
            out_avals.append(
                jax.core.ShapedArray(
                    tuple(alloc.tensor_shape), mybir.dt.np(alloc.dtype)
                )
            )
    n_params = len(in_names)
    all_in_names = list(in_names)
    if part_name is not None:
        all_in_names = all_in_names + [part_name]

    def _body(*args):
        operands = list(args)
        if part_name is not None:
            operands.append(b2j.partition_id_tensor())
        outs = b2j._bass_exec_p.bind(
            *operands,
            out_avals=tuple(out_avals),
            in_names=tuple(all_in_names),
            out_names=tuple(out_names),
            lowering_input_output_aliases=(),
            sim_require_finite=True,
            sim_require_nnan=True,
            nc=nc,
        )
        return tuple(outs)

    if n_cores == 1:
        fn = jax.jit(_body, keep_unused=True)
    else:
        devices = jax.devices()[:n_cores]
        mesh = Mesh(np.asarray(devices), ("core",))
        fn = jax.jit(
            shard_map(
                _body,
                mesh=mesh,
                in_specs=(PartitionSpec("core"),) * n_params,
                out_specs=(PartitionSpec("core"),) * len(out_names),
                check_rep=False,
            ),
            keep_unused=True,
        )
    _EXEC_CACHE[n_cores] = (fn, in_names, out_names)
    return _EXEC_CACHE[n_cores]


def _run(nc, in_maps, n_cores):
    """Execute via PJRT. Unlike run_bass_via_pjrt, NO initializing operand
    is passed for the ExternalOutput: the kernel writes every element of
    `out`, so the result buffer needs no zero-fill, and skipping the
    operand avoids re-staging its bytes on every execution."""
    fn, in_names, out_names = _get_exec(nc, n_cores)
    per_core = [[m[nm] for nm in in_names] for m in in_maps]
    if n_cores == 1:
        out_arrs = fn(*per_core[0])
        return [{nm: np.asarray(out_arrs[i]) for i, nm in enumerate(out_names)}]

    concat = [
        np.concatenate([per_core[c][i] for c in range(n_cores)], axis=0)
        for i in range(len(in_names))
    ]
    out_arrs = fn(*concat)
    res = []
    for c in range(n_cores):
        d = {}
        for i, nm in enumerate(out_names):
            full = np.asarray(out_arrs[i])
            sh0 = full.shape[0] // n_cores
            d[nm] = full[c * sh0 : (c + 1) * sh0]
        res.append(d)
    return res


def kernel(prev_layer_output, input_A_weights, input_B_weights, table_weights):
    nc = _get_nc(N_CORES)
    in_maps = make_in_maps(
        prev_layer_output, input_A_weights, input_B_weights, table_weights, N_CORES
    )
    res = _run(nc, in_maps, N_CORES)
    out = np.concatenate([res[c]["out"] for c in range(N_CORES)], axis=1)
    return out.astype(np.float32)

